# revision 26
# baseline (speedup 1.0000x reference)
"""Trainium2 Bass kernel for nn_GBSTokenizer.

Math: the reference's route softmax is over a size-1 axis, so the route
probabilities are exactly 1.0 and the L x L calibration matmul collapses to a
scalar ~1 (verified |s-1| < 6e-8, output deviation < 6e-7 absmax-relative).
The computation therefore reduces to

    out = poolsum(conv1d(X, conv_w) + conv_b) @ wd + bd

where poolsum(z)[l] = z[l] + mean2[l//2] + mean3[l//3] + mean4[l//4]
(multi-scale block means, blocks of size 2/3/4). Since everything between
the conv and the final projection is linear, wd is folded into the conv
weights on the host:  Wk_eff = conv_w[:,:,k].T @ wd, so the device computes

    Z[dout, l] = sum_k  Wk_eff[k].T @ XT[:, l+k-1]
    S = poolsum_over_l(Z) + const                          (vector ops)
    out[l, :]  = S[:, l]                                   (host transpose)

with const = 4*(conv_b @ wd) + bd added as a per-partition bias at PSUM
eviction (divided by 4 because poolsum multiplies constants by exactly 4).

Matmul precision: fp8(e4m3) DoubleRow with hi/lo error compensation.
Both W and X are split host-side into fp8 pairs at a shared power-of-2
scale (W*64 = Wh + Wl, X*16 = Xh + Xl, residuals stored at the same
scale so everything accumulates in one PSUM group):

    Z ~= (Xh@Wh + Xh@Wl + Xl@Wh) / 1024

Each DoubleRow matmul contracts 2 x 128 at 0.5 cycles/output column, so
the 3-term scheme costs 13.5 cyc/col vs 18 for bf16 while landing at
~1e-3 absmax-rel error (measured on the fixed inputs; gate is 2e-2).

Pooling combine per 408-col chunk (divisible by 12 = lcm(2,3,4), so all
blocks are chunk-local), with sum2[j] = pairsum, sum3[t] = triplesum:
    W [i] = 3*sum2[2i] +   sum2[2i+1]
    W'[i] =   sum2[2i] + 3*sum2[2i+1]
    S[4i+{0,1}] = Z + 0.25*W[i];  S[4i+{2,3}] = Z + 0.25*W'[i]
    S[3t+p]    += (1/3)*sum3[t]
The final combine writes bf16 (halves the output DMA).

Sharding: data-parallel over batch N=8, one sample per NeuronCore, params
replicated. All compute layouts keep feature dim on partitions and sequence
dim on the free axis (transposed), so the host transposes X in and out.
"""

import numpy as np
import ml_dtypes

# Problem shape (hardcoded per harness contract).
N_SAMPLES = 8
L = 2040
D = 768
NT = D // 128          # 6 partition tiles over features
NCH = 5                # l-chunks per psum pass
CHW = L // NCH         # 408 columns per chunk (<=512 fp32 = 1 PSUM bank),
                       # divisible by 12 so pooling is chunk-local
HALF = CHW // 2        # DoubleRow moving free = 2*cols; keep 2*204 <= 512
NKT = 3 * NT           # 18 k-tiles (di-major: idx = di*3 + k)
N_CORES = 8

SW = 64.0              # weight fp8 scale
SX = 16.0              # activation fp8 scale
BF16 = ml_dtypes.bfloat16
E4M3 = ml_dtypes.float8_e4m3

_CACHE = {}


def _build_bass():
    import concourse.bacc as bacc
    import concourse.bass as bass
    import concourse.tile as tile
    from concourse import mybir

    def bcast(ap2d, k):
        # Append a step-0 (broadcast) innermost dim to a 2D AP.
        return bass.AP(tensor=ap2d.tensor, offset=ap2d.offset,
                       ap=[*list(ap2d.ap), [0, k]])

    f32 = mybir.dt.float32
    bf16 = mybir.dt.bfloat16
    fp8 = mybir.dt.float8e4
    Alu = mybir.AluOpType
    DR = mybir.MatmulPerfMode.DoubleRow

    nc = bacc.Bacc(
        "TRN2", target_bir_lowering=False, debug=False, num_devices=N_CORES)
    # xq: X^T laid out [partition, col, di, hi/lo] — di and the hi/lo fp8
    # planes innermost.  Any DoubleRow slot pair (cross-di or hi/lo) then
    # spans a narrow byte interval, so the interval-based dependency
    # tracker ties each matmul only to its own column range's DMA piece;
    # pieces are >=4.8KB contiguous rows (no small-descriptor penalty).
    # Zero-padded halo col on each side.
    xq_d = nc.dram_tensor("xq", [128, (L + 2) * 2 * NT], fp8,
                          kind="ExternalInput")
    # wc: per dout-tile, slot-major: [slot][ktile][dout] with slot0 = Wl,
    # slot1 = Wh (so the hot slot1 plane can be DMA'd first).
    wc_d = nc.dram_tensor("wc", [NT, 128, 2 * NKT * 128], fp8,
                          kind="ExternalInput")
    c4_d = nc.dram_tensor("c4", [128, NT], f32, kind="ExternalInput")
    out_d = nc.dram_tensor("out", [D, L], bf16, kind="ExternalOutput")

    # xq DMA piece boundaries: one chunk + conv halo per piece (chunk 0
    # split in half so the first matmuls can start sooner).
    xcuts = [0, 206, 410, 818, 1226, 1634, L + 2]
    N_WARM = 31            # PE warm-up matmuls (p-state ramp cover)

    with tile.TileContext(nc) as tc:
        with (
            tc.tile_pool(name="const", bufs=1) as cpool,
            tc.tile_pool(name="ztmp", bufs=3) as zpool,
            tc.tile_pool(name="ptmp", bufs=3) as tpool,
            tc.tile_pool(name="psum", bufs=1, space="PSUM") as ppool,
        ):
            xq = cpool.tile([128, L + 2, NT, 2], fp8, tag="xq")
            wc = cpool.tile([128, NT, 2, NKT, 128], fp8, tag="wc")
            c4 = cpool.tile([128, NT], f32, tag="c4")
            zb = cpool.tile([128, NT, L], bf16, tag="zb")

            wcv = wc_d.rearrange("t p (s k m) -> t p s k m", s=2, m=128)
            CSTR = 2 * NT  # bytes per column in xq (di x hi/lo)

            def xq_piece(c0, c1):
                nc.sync.dma_start(out=xq[:, c0:c1, :, :],
                                  in_=xq_d[:, CSTR * c0:CSTR * c1])

            # PE warm-up: matmuls on a memset scratch keep the tensor engine
            # continuously busy from t~0 so the p-state ramp completes while
            # the startup DMAs are in flight (PE dispatch is by readiness, so
            # real matmuls seamlessly take over as their data lands).
            warm = cpool.tile([128, 128], bf16, tag="warm")
            wps = ppool.tile([128, 128], f32, name="wps", tag="wps")
            nc.vector.memset(warm, 0.0)
            for i in range(N_WARM):
                nc.tensor.matmul(wps, warm, warm,
                                 start=(i == 0), stop=(i == N_WARM - 1))

            # DMA emission order = priority. dt pairs are interleaved in the
            # compute loop, so each xq chunk piece feeds two units (~4.6us of
            # PE work per ~1.7us piece) and the bus stays ahead of the PE.
            nc.sync.dma_start(out=wc[:, 0, 1], in_=wcv[0][:, 1])
            xq_piece(xcuts[0], xcuts[1])
            xq_piece(xcuts[1], xcuts[2])
            nc.sync.dma_start(out=wc[:, 0, 0], in_=wcv[0][:, 0])
            nc.sync.dma_start(out=wc[:, 1], in_=wcv[1])
            xq_piece(xcuts[2], xcuts[3])
            nc.sync.dma_start(out=c4[:, :], in_=c4_d[:, :])
            for c in range(3, 6):
                xq_piece(xcuts[c], xcuts[c + 1])
            for t in range(2, NT):
                nc.sync.dma_start(out=wc[:, t], in_=wcv[t])

            pstr = xq.ap[0][0]  # partition stride

            def xq_off(pl, di, col):
                return xq.offset + col * CSTR + di * 2 + pl

            units = [(2 * dp + i, c)
                     for dp in range(NT // 2)
                     for c in range(NCH)
                     for i in range(2)]
            for ui, (dt, c) in enumerate(units):
                    psc = ppool.tile([128, CHW], f32, name=f"ps{ui % 7}",
                                     tag=f"ps{ui % 7}")
                    first = True
                    for h in range(2):
                        base = c * CHW + h * HALF
                        # main pass: slot pair = k-tiles (2t, 2t+1) of Wh
                        for t in range(NKT // 2):
                            di0, k0 = divmod(2 * t, 3)
                            di1, k1 = divmod(2 * t + 1, 3)
                            delta = (k1 - k0) * CSTR + (di1 - di0) * 2
                            rhs = bass.AP(
                                tensor=xq.tensor,
                                offset=xq_off(0, di0, base + k0),
                                ap=[[pstr, 128], [delta, 2], [CSTR, HALF]])
                            nc.tensor.matmul(
                                psc[:, h * HALF:(h + 1) * HALF],
                                wc[:, dt, 1, 2 * t:2 * t + 2, :],
                                rhs,
                                start=first, stop=False, perf_mode=DR)
                            first = False
                        # correction pass: slots (Wl, Xh), (Wh, Xl) per k-tile
                        for t in range(NKT):
                            di, k = divmod(t, 3)
                            rhs = bass.AP(
                                tensor=xq.tensor,
                                offset=xq_off(0, di, base + k),
                                ap=[[pstr, 128], [1, 2], [CSTR, HALF]])
                            nc.tensor.matmul(
                                psc[:, h * HALF:(h + 1) * HALF],
                                wc[:, dt, 0:2, t, :],
                                rhs,
                                start=False,
                                stop=(h == 1 and t == NKT - 1),
                                perf_mode=DR)

                    zh = zpool.tile([128, CHW], f32, name="zh", tag="zh")
                    nc.scalar.activation(
                        out=zh,
                        in_=psc,
                        func=mybir.ActivationFunctionType.Identity,
                        bias=c4[:, dt:dt + 1],
                        scale=1.0 / (SW * SX),
                    )

                    # ---- multi-scale pooling, chunk-local (CHW % 12 == 0) ----
                    z2 = zh.rearrange("p (n two) -> p n two", two=2)
                    z3 = zh.rearrange("p (n three) -> p n three", three=3)
                    z4 = zh.rearrange("p (n four) -> p n four", four=4)
                    zb3 = zb[:, dt, c * CHW:(c + 1) * CHW].rearrange(
                        "p (n three) -> p n three", three=3)

                    sum2 = tpool.tile([128, CHW // 2], f32, name="sum2",
                                      tag="sum2")
                    sum3 = tpool.tile([128, CHW // 3], f32, name="sum3",
                                      tag="sum3")
                    wab = tpool.tile([128, CHW // 2], f32, name="wab",
                                     tag="wab")
                    s2v = sum2.rearrange("p (n two) -> p n two", two=2)
                    wv2 = wab.rearrange("p (n two) -> p n two", two=2)

                    # sum2 on DVE (heads the W -> S2 chain), sum3 on GPSIMD
                    # in parallel: shortens the per-chunk critical path.
                    nc.vector.tensor_add(sum2, z2[:, :, 0], z2[:, :, 1])
                    nc.gpsimd.tensor_add(sum3, z3[:, :, 0], z3[:, :, 1])
                    nc.gpsimd.tensor_add(sum3, sum3, z3[:, :, 2])
                    # W / W' in ONE op: out[i,j] = 3*sum2[2i+j] + sum2[2i+1-j]
                    # via a swapped-pair (negative-step) view of sum2.
                    swp = bass.AP(tensor=sum2.tensor,
                                  offset=s2v[:, :, 1].offset,
                                  ap=[*list(s2v[:, :, 1].ap), [-1, 2]])
                    nc.vector.scalar_tensor_tensor(
                        wv2, s2v, 3.0, swp, Alu.mult, Alu.add)
                    # S = Z + 0.25 * rep(W|W'): positions 4i+{0,1} get W[i],
                    # 4i+{2,3} get W'[i]; step-0 APs broadcast W over the pair.
                    nc.vector.scalar_tensor_tensor(
                        z4[:, :, 0:2], bcast(wv2[:, :, 0], 2), 0.25,
                        z4[:, :, 0:2], Alu.mult, Alu.add)
                    nc.vector.scalar_tensor_tensor(
                        z4[:, :, 2:4], bcast(wv2[:, :, 1], 2), 0.25,
                        z4[:, :, 2:4], Alu.mult, Alu.add)
                    # S += rep3(sum3)/3 in one op, writing the bf16 output
                    # buffer (sum3 broadcast over triples).
                    nc.vector.scalar_tensor_tensor(
                        zb3, bcast(sum3, 3), 1.0 / 3.0, z3,
                        Alu.mult, Alu.add)

                    # ship finished output early; the final piece is small so
                    # the post-compute tail is short
                    if c == 1:
                        nc.sync.dma_start(
                            out=out_d[dt * 128:(dt + 1) * 128, 0:2 * CHW],
                            in_=zb[:, dt, 0:2 * CHW])
                    elif c == 3:
                        nc.sync.dma_start(
                            out=out_d[dt * 128:(dt + 1) * 128, 2 * CHW:4 * CHW],
                            in_=zb[:, dt, 2 * CHW:4 * CHW])
                    elif c == 4:
                        nc.sync.dma_start(
                            out=out_d[dt * 128:(dt + 1) * 128, 4 * CHW:L],
                            in_=zb[:, dt, 4 * CHW:L])

    nc.compile()
    return nc


def _get_nc():
    if "nc" not in _CACHE:
        _CACHE["nc"] = _build_bass()
    return _CACHE["nc"]


def _q8(a):
    return a.astype(E4M3)


def _prep_host(X, conv_w, conv_b, wd, bd):
    """Fold wd into conv weights; fp8 hi/lo split; per-core transposed X."""
    # Wk_eff[k] = conv_w[:,:,k].T @ wd   (fp64), scaled by SW, split hi/lo.
    wc = np.empty((NT, 128, 2, NKT, 128), dtype=E4M3)
    for k in range(3):
        we = (conv_w[:, :, k].T.astype(np.float64)
              @ wd.astype(np.float64)) * SW        # [din, dout]
        wh = _q8(we.astype(np.float32))
        wl = _q8((we - wh.astype(np.float64)).astype(np.float32))
        # k-tile t = di*3 + k holds din block di; slot0 = Wl, slot1 = Wh.
        wh4 = wh.reshape(NT, 128, NT, 128)   # [di, p, dt, m]
        wl4 = wl.reshape(NT, 128, NT, 128)
        for di in range(NT):
            t = di * 3 + k
            wc[:, :, 0, t, :] = wl4[di].transpose(1, 0, 2)  # [dt, p, m]
            wc[:, :, 1, t, :] = wh4[di].transpose(1, 0, 2)
    wc = np.ascontiguousarray(wc.reshape(NT, 128, 2 * NKT * 128))

    const = 4.0 * (conv_b.astype(np.float64) @ wd.astype(np.float64)) \
        + bd.astype(np.float64)
    c4 = (const / 4.0).astype(np.float32).reshape(NT, 128).T.copy()

    xqs = []
    for n in range(X.shape[0]):
        xt = np.zeros((D, L + 2), dtype=np.float32)
        xt[:, 1:L + 1] = X[n].T * SX
        xh = _q8(xt)
        xl = _q8(xt - xh.astype(np.float32))
        xq = np.empty((128, L + 2, NT, 2), dtype=E4M3)
        xq[:, :, :, 0] = xh.reshape(NT, 128, L + 2).transpose(1, 2, 0)
        xq[:, :, :, 1] = xl.reshape(NT, 128, L + 2).transpose(1, 2, 0)
        xqs.append(xq.reshape(128, (L + 2) * 2 * NT))
    return xqs, wc, c4


def _get_runner():
    """Cached jitted SPMD executor (mirrors bass2jax.run_bass_via_pjrt)."""
    if "runner" in _CACHE:
        return _CACHE["runner"]

    import jax
    import jax.numpy as jnp  # noqa: F401
    from jax.experimental.shard_map import shard_map
    from jax.sharding import Mesh, PartitionSpec
    import concourse.mybir as mybir
    from concourse import bass2jax

    nc = _get_nc()
    bass2jax.install_neuronx_cc_hook()

    part_name = nc.partition_id_tensor.name if nc.partition_id_tensor else None
    in_names, out_names, out_avals = [], [], []
    for alloc in nc.m.functions[0].allocations:
        if not isinstance(alloc, mybir.MemoryLocationSet):
            continue
        name = alloc.memorylocations[0].name
        if alloc.kind == "ExternalInput":
            if name != part_name:
                in_names.append(name)
        elif alloc.kind == "ExternalOutput":
            out_names.append(name)
            out_avals.append(jax.core.ShapedArray(
                tuple(alloc.tensor_shape), mybir.dt.np(alloc.dtype)))
    n_params = len(in_names)
    all_names = tuple(
        in_names + out_names + ([part_name] if part_name else []))

    def _body(*args):
        operands = list(args)
        if part_name is not None:
            operands.append(bass2jax.partition_id_tensor())
        outs = bass2jax._bass_exec_p.bind(
            *operands,
            out_avals=tuple(out_avals),
            in_names=all_names,
            out_names=tuple(out_names),
            lowering_input_output_aliases=(),
            sim_require_finite=True,
            sim_require_nnan=True,
            nc=nc,
        )
        return tuple(outs)

    devices = jax.devices()[:N_CORES]
    mesh = Mesh(np.asarray(devices), ("core",))
    n_outs = len(out_names)
    sharded = jax.jit(
        shard_map(_body, mesh=mesh,
                  in_specs=(PartitionSpec("core"),) * (n_params + n_outs),
                  out_specs=(PartitionSpec("core"),) * n_outs,
                  check_rep=False),
        donate_argnums=tuple(range(n_params, n_params + n_outs)),
        keep_unused=True,
    )
    # Device-side zero buffers for the donated outputs (avoids shipping
    # N_CORES * MBs of zeros through the tunnel every call).
    from jax.sharding import NamedSharding
    make_zeros = [
        jax.jit(
            (lambda shape, dtype: (lambda: jnp.zeros(shape, dtype)))(
                (N_CORES * a.shape[0], *a.shape[1:]), a.dtype),
            out_shardings=NamedSharding(mesh, PartitionSpec("core")))
        for a in out_avals
    ]
    _CACHE["runner"] = (sharded, in_names, out_names, out_avals, make_zeros)
    return _CACHE["runner"]


def kernel(**inputs):
    X = np.asarray(inputs["X"], dtype=np.float32)
    conv_w = np.asarray(inputs["conv_w"], dtype=np.float32)
    conv_b = np.asarray(inputs["conv_b"], dtype=np.float32)
    wd = np.asarray(inputs["wd"], dtype=np.float32)
    bd = np.asarray(inputs["bd"], dtype=np.float32)

    xqs, wc_host, c4 = _prep_host(X, conv_w, conv_b, wd, bd)

    res = None
    for attempt in range(3):
        try:
            sharded, in_names, out_names, out_avals, make_zeros = _get_runner()
            per_core = {"xq": xqs, "wc": [wc_host] * N_CORES,
                        "c4": [c4] * N_CORES}
            concat_in = [np.concatenate(per_core[nm], axis=0)
                         for nm in in_names]
            concat_zeros = [mz() for mz in make_zeros]
            out_arrs = sharded(*concat_in, *concat_zeros)
            res = np.asarray(out_arrs[out_names.index("out")])
            break
        except Exception:
            # Transient device wedge (can be inherited from a previous
            # crashed process on the shared terminal). Reset the PJRT
            # client and rebuild the jitted runner, then retry.
            if attempt == 2:
                raise
            import time
            import jax
            import jax._src.xla_bridge as _xb
            time.sleep(5.0)
            _CACHE.pop("runner", None)
            try:
                jax.clear_caches()
                _xb._clear_backends()
            except Exception:
                pass
    res = res.reshape(N_CORES, D, L).astype(np.float32)

    out = np.empty((N_SAMPLES, L, D), dtype=np.float32)
    for n in range(N_SAMPLES):
        out[n] = res[n].T
    return out
